# revision 3
# baseline (speedup 1.0000x reference)
"""HardBinaryConv forward on 8 trn2 NeuronCores.

y = conv2d(sign(x), scaling[o] * sign(w)), 3x3, pad 1, stride 1
  x: (32, 256, 56, 56) f32, w: (256, 256, 3, 3) f32
  scaling[o] = mean(|w[o]|)

Strategy: data-parallel over batch (4 images / core), weights replicated.
Per core the conv is computed as 18 accumulating bf16 matmuls per output
tile (9 taps x 2 input-channel groups): PSUM[o,pos] += sign(w)[i,o].T @
sign(x)[i, pos+tap_offset], on a horizontally+vertically padded (58x58)
image layout so every tap is a flat offset into the same buffer.
sign values are exact in bf16 and PSUM accumulates fp32, so the conv
result is exact; final per-channel scale applied on VectorE.
"""

import numpy as np

import concourse.bass as bass
import concourse.mybir as mybir
import concourse.tile as tile
from concourse import bacc
from concourse.bass import ds
from concourse.bass_utils import run_bass_kernel_spmd

N_CORES = 8
NIMG = 4            # images per core (32 / 8)
CIN = 256
COUT = 256
IG = 2              # input-channel groups of 128
OG = 2              # output-channel groups of 128
H = W = 56
HP = WP = 58        # padded spatial
PPI = HP * WP       # 3364 padded positions per image
NPOS = H * W        # 3136
T = 9               # 3x3 taps
RROWS = 8           # padded rows per output tile
RT = 7              # row tiles per image (rows 1..56)
NTILE = RROWS * WP  # 464 matmul free dim
NBLK = 1 + IG * NIMG  # leading dummy block absorbs tap offset -1 reads
BA_LEN = NBLK * PPI + 64

F32 = mybir.dt.float32
BF16 = mybir.dt.bfloat16

_CACHE = {}


def build_nc():
    nc = bacc.Bacc("TRN2", target_bir_lowering=False, debug=False,
                   num_devices=N_CORES)

    x_d = nc.declare_dram_parameter("x", [NIMG, CIN, NPOS], F32, isOutput=False)
    wn_d = nc.declare_dram_parameter("w_nat", [OG, 128, CIN * T], F32,
                                     isOutput=False)
    wt_d = nc.declare_dram_parameter("w_t", [T, IG, 128, COUT], F32,
                                     isOutput=False)
    y_d = nc.declare_dram_parameter("y", [NIMG, COUT, NPOS], F32, isOutput=True)

    with tile.TileContext(nc) as tc:
        with (
            tc.tile_pool(name="persist", bufs=1) as persist,
            tc.tile_pool(name="xs", bufs=3) as xsp,
            tc.tile_pool(name="yb", bufs=4) as ybp,
            tc.tile_pool(name="ps", bufs=4, space=bass.MemorySpace.PSUM) as psp,
        ):
            ba = persist.tile([128, BA_LEN], BF16)
            ba4 = ba[:, 0:NBLK * PPI].rearrange(
                "p (b h w) -> p b h w", b=NBLK, h=HP, w=WP)
            wsign = persist.tile([128, T * IG, COUT], BF16)
            wstage = persist.tile([128, T * IG, COUT], F32)
            wnstage = persist.tile([128, OG, CIN * T], F32)
            scal = persist.tile([128, OG], F32)

            # binarized weights: lhsT tiles [i_in, o] per (tap, i_grp)
            for t in range(T):
                for g in range(IG):
                    nc.sync.dma_start(out=wstage[:, t * IG + g, :],
                                      in_=wt_d[t, g])
            nc.scalar.sign(wsign[:, :, :], wstage[:, :, :])

            # per-output-channel scaling = mean |w| over (i, kh, kw)
            for q in range(OG):
                nc.sync.dma_start(out=wnstage[:, q, :], in_=wn_d[q])
                nc.vector.tensor_reduce(
                    out=scal[:, q:q + 1], in_=wnstage[:, q, :],
                    axis=mybir.AxisListType.X, op=mybir.AluOpType.add,
                    apply_absolute_value=True)
            nc.vector.tensor_scalar_mul(scal[:, :], scal[:, :],
                                        1.0 / (CIN * T))

            # zero padding borders (+ margins that feed discarded columns)
            nc.vector.memset(ba4[:, 0, HP - 1, :], 0.0)
            nc.vector.memset(ba[:, NBLK * PPI:], 0.0)
            nc.vector.memset(ba4[:, 1:NBLK, 0, :], 0.0)
            nc.vector.memset(ba4[:, 1:NBLK, HP - 1, :], 0.0)
            nc.vector.memset(ba4[:, 1:NBLK, :, 0], 0.0)
            nc.vector.memset(ba4[:, 1:NBLK, :, WP - 1], 0.0)

            # binarized activations into padded layout
            for n in range(NIMG):
                for g in range(IG):
                    xs = xsp.tile([128, NPOS], F32)
                    nc.sync.dma_start(out=xs[:, :],
                                      in_=x_d[n, g * 128:(g + 1) * 128, :])
                    blk = 1 + g * NIMG + n
                    nc.scalar.sign(
                        ba4[:, blk, 1:H + 1, 1:W + 1],
                        xs[:, :].rearrange("p (h w) -> p h w", h=H, w=W))

            # main conv loop
            for n in range(NIMG):
                for r in range(RT):
                    h0 = 1 + r * RROWS
                    for q in range(OG):
                        ps = psp.tile([128, NTILE], F32)
                        k = 0
                        for t in range(T):
                            kh, kw = t // 3, t % 3
                            for g in range(IG):
                                blk = 1 + g * NIMG + n
                                off = blk * PPI + (h0 + kh - 1) * WP + (kw - 1)
                                nc.tensor.matmul(
                                    ps[:, :],
                                    wsign[:, t * IG + g,
                                          q * 128:(q + 1) * 128],
                                    ba[:, ds(off, NTILE)],
                                    start=(k == 0), stop=(k == T * IG - 1))
                                k += 1
                        yb = ybp.tile([128, RROWS, W], F32)
                        ps3 = ps[:, :].rearrange("p (h w) -> p h w",
                                                 h=RROWS, w=WP)
                        nc.vector.tensor_scalar_mul(
                            yb[:, :, :], ps3[:, :, 1:W + 1], scal[:, q:q + 1])
                        nc.sync.dma_start(
                            out=y_d[n, q * 128:(q + 1) * 128,
                                    ds((h0 - 1) * W, RROWS * W)],
                            in_=yb[:, :, :])

    nc.compile()
    return nc


def _get_nc():
    if "nc" not in _CACHE:
        _CACHE["nc"] = build_nc()
    return _CACHE["nc"]


def _prep_inputs(x, weight):
    x = np.ascontiguousarray(x, dtype=np.float32)
    weight = np.ascontiguousarray(weight, dtype=np.float32)
    # natural layout for the |w| reduction: (o_grp, o_in, i*9)
    w_nat = weight.reshape(OG, 128, CIN * T)
    # transposed layout for lhsT tiles: (tap, i_grp, i_in, o)
    w_t = np.ascontiguousarray(
        weight.transpose(2, 3, 1, 0).reshape(T, IG, 128, COUT))
    in_maps = []
    for c in range(N_CORES):
        xs = x[c * NIMG:(c + 1) * NIMG].reshape(NIMG, CIN, NPOS)
        in_maps.append({"x": np.ascontiguousarray(xs),
                        "w_nat": w_nat, "w_t": w_t})
    return in_maps


def _assemble(results):
    parts = [results[c]["y"].reshape(NIMG, COUT, H, W) for c in range(N_CORES)]
    return np.concatenate(parts, axis=0)


def run(x, weight, **run_kwargs):
    nc = _get_nc()
    in_maps = _prep_inputs(x, weight)
    res = run_bass_kernel_spmd(nc, in_maps, list(range(N_CORES)), **run_kwargs)
    return _assemble(res.results), res


def kernel(x, weight):
    y, _ = run(x, weight)
    return y


# revision 4
# speedup vs baseline: 1.0218x; 1.0218x over previous
"""HardBinaryConv forward on 8 trn2 NeuronCores.

y = conv2d(sign(x), scaling[o] * sign(w)), 3x3, pad 1, stride 1
  x: (32, 256, 56, 56) f32, w: (256, 256, 3, 3) f32
  scaling[o] = mean(|w[o]|)

Strategy: data-parallel over batch (4 images / core), weights replicated.
Per core the conv is computed as 18 accumulating bf16 matmuls per output
tile (9 taps x 2 input-channel groups): PSUM[o,pos] += sign(w)[i,o].T @
sign(x)[i, pos+tap_offset], on a fully padded (58x58) image layout so
every tap is a flat offset into the same buffer. sign values are exact
in bf16 and PSUM accumulates fp32, so the conv is exact; the final
per-channel scale is applied on VectorE during PSUM eviction.

Engine/queue split: x loads + sign on ScalarE (HW DMA queue), weights +
y stores on SyncE, scaling-path loads on GpSimd, scale-evict on VectorE,
so DMA triggers never serialize behind each other on one queue.
"""

import numpy as np

import concourse.bass as bass
import concourse.mybir as mybir
import concourse.tile as tile
from concourse import bacc
from concourse.bass import ds
from concourse.bass_utils import run_bass_kernel_spmd

N_CORES = 8
NIMG = 4            # images per core (32 / 8)
CIN = 256
COUT = 256
IG = 2              # input-channel groups of 128
OG = 2              # output-channel groups of 128
H = W = 56
HP = WP = 58        # padded spatial
PPI = HP * WP       # 3364 padded positions per image
NPOS = H * W        # 3136
T = 9               # 3x3 taps
RROWS = 8           # padded rows per output tile
RT = 7              # row tiles per image (rows 1..56)
NTILE = RROWS * WP  # 464 matmul free dim
NBLK = 1 + NIMG     # per-group: dummy margin block + 4 images
GLEN = NBLK * PPI + 64   # per-group flat length incl tail margin
XCH = 2             # x DMA/sign chunks per (image, group)
CROWS = H // XCH    # 28 rows per chunk

F32 = mybir.dt.float32
BF16 = mybir.dt.bfloat16

_CACHE = {}


def build_nc():
    nc = bacc.Bacc("TRN2", target_bir_lowering=False, debug=False,
                   num_devices=N_CORES)

    x_d = nc.declare_dram_parameter("x", [NIMG, CIN, NPOS], F32, isOutput=False)
    wn_d = nc.declare_dram_parameter("w_nat", [OG, 128, CIN * T], F32,
                                     isOutput=False)
    wt_d = nc.declare_dram_parameter("w_t", [T, IG, 128, COUT], F32,
                                     isOutput=False)
    y_d = nc.declare_dram_parameter("y", [NIMG, COUT, NPOS], F32, isOutput=True)

    with tile.TileContext(nc) as tc:
        with (
            tc.tile_pool(name="persist", bufs=1) as persist,
            tc.tile_pool(name="xs", bufs=6) as xsp,
            tc.tile_pool(name="yb", bufs=4) as ybp,
            tc.tile_pool(name="ps", bufs=6, space=bass.MemorySpace.PSUM) as psp,
        ):
            ba = persist.tile([128, IG, GLEN], BF16)
            wsign = persist.tile([128, T * IG, COUT], BF16)
            wstage = persist.tile([128, T * IG, COUT], F32)
            wnstage = persist.tile([128, OG, CIN * T], F32)
            scal = persist.tile([128, OG], F32)

            # binarized weights: lhsT tiles [i_in, o] per (tap, i_grp);
            # one strided DMA so the trigger doesn't serialize 18x
            nc.sync.dma_start(
                out=wstage[:, :, :],
                in_=wt_d[:, :, :, :].rearrange("t g p o -> p (t g) o"))
            nc.scalar.sign(wsign[:, :, :], wstage[:, :, :])

            # zero padding borders (+ margins feeding discarded columns)
            ba4 = ba[:, :, 0:NBLK * PPI].rearrange(
                "p g (b h w) -> p g b h w", b=NBLK, h=HP, w=WP)
            nc.vector.memset(ba4[:, :, 0, HP - 1, :], 0.0)   # dummy tail row
            nc.vector.memset(ba[:, :, NBLK * PPI:], 0.0)     # tail margin
            nc.vector.memset(ba4[:, :, 1:NBLK, 0, :], 0.0)
            nc.vector.memset(ba4[:, :, 1:NBLK, HP - 1, :], 0.0)
            nc.vector.memset(ba4[:, :, 1:NBLK, :, 0], 0.0)
            nc.vector.memset(ba4[:, :, 1:NBLK, :, WP - 1], 0.0)

            # binarized activations, row-chunked so image 0 is ready fast
            for n in range(NIMG):
                for c in range(XCH):
                    for g in range(IG):
                        xs = xsp.tile([128, CROWS * W], F32)
                        nc.scalar.dma_start(
                            out=xs[:, :],
                            in_=x_d[n, g * 128:(g + 1) * 128,
                                    ds(c * CROWS * W, CROWS * W)])
                        r0 = 1 + c * CROWS
                        nc.scalar.sign(
                            ba4[:, g, n + 1, r0:r0 + CROWS, 1:W + 1],
                            xs[:, :].rearrange("p (h w) -> p h w",
                                               h=CROWS, w=W))

            # per-output-channel scaling = mean |w| over (i, kh, kw)
            for q in range(OG):
                nc.gpsimd.dma_start(out=wnstage[:, q, :], in_=wn_d[q])
                nc.vector.tensor_reduce(
                    out=scal[:, q:q + 1], in_=wnstage[:, q, :],
                    axis=mybir.AxisListType.X, op=mybir.AluOpType.add,
                    apply_absolute_value=True)
            nc.vector.tensor_scalar_mul(scal[:, :], scal[:, :],
                                        1.0 / (CIN * T))

            # main conv loop
            for n in range(NIMG):
                for r in range(RT):
                    h0 = 1 + r * RROWS
                    for q in range(OG):
                        ps = psp.tile([128, NTILE], F32)
                        k = 0
                        for t in range(T):
                            kh, kw = t // 3, t % 3
                            for g in range(IG):
                                off = ((n + 1) * PPI
                                       + (h0 + kh - 1) * WP + (kw - 1))
                                nc.tensor.matmul(
                                    ps[:, :],
                                    wsign[:, t * IG + g,
                                          q * 128:(q + 1) * 128],
                                    ba[:, g, ds(off, NTILE)],
                                    start=(k == 0), stop=(k == T * IG - 1))
                                k += 1
                        yb = ybp.tile([128, RROWS, W], F32)
                        ps3 = ps[:, :].rearrange("p (h w) -> p h w",
                                                 h=RROWS, w=WP)
                        nc.vector.tensor_scalar_mul(
                            yb[:, :, :], ps3[:, :, 1:W + 1], scal[:, q:q + 1])
                        nc.sync.dma_start(
                            out=y_d[n, q * 128:(q + 1) * 128,
                                    ds((h0 - 1) * W, RROWS * W)],
                            in_=yb[:, :, :])

    nc.compile()
    return nc


def _get_nc():
    if "nc" not in _CACHE:
        _CACHE["nc"] = build_nc()
    return _CACHE["nc"]


def _prep_inputs(x, weight):
    x = np.ascontiguousarray(x, dtype=np.float32)
    weight = np.ascontiguousarray(weight, dtype=np.float32)
    # natural layout for the |w| reduction: (o_grp, o_in, i*9)
    w_nat = weight.reshape(OG, 128, CIN * T)
    # transposed layout for lhsT tiles: (tap, i_grp, i_in, o)
    w_t = np.ascontiguousarray(
        weight.transpose(2, 3, 1, 0).reshape(T, IG, 128, COUT))
    in_maps = []
    for c in range(N_CORES):
        xs = x[c * NIMG:(c + 1) * NIMG].reshape(NIMG, CIN, NPOS)
        in_maps.append({"x": np.ascontiguousarray(xs),
                        "w_nat": w_nat, "w_t": w_t})
    return in_maps


def _assemble(results):
    parts = [results[c]["y"].reshape(NIMG, COUT, H, W) for c in range(N_CORES)]
    return np.concatenate(parts, axis=0)


def run(x, weight, **run_kwargs):
    nc = _get_nc()
    in_maps = _prep_inputs(x, weight)
    res = run_bass_kernel_spmd(nc, in_maps, list(range(N_CORES)), **run_kwargs)
    return _assemble(res.results), res


def kernel(x, weight):
    y, _ = run(x, weight)
    return y


# revision 5
# speedup vs baseline: 1.5591x; 1.5259x over previous
"""HardBinaryConv forward on 8 trn2 NeuronCores.

y = conv2d(sign(x), scaling[o] * sign(w)), 3x3, pad 1, stride 1
  x: (32, 256, 56, 56) f32, w: (256, 256, 3, 3) f32
  scaling[o] = mean(|w[o]|)

Strategy: data-parallel over batch (4 images / core), weights replicated.
Per core the conv is computed as 9 accumulating fp8 DoubleRow matmuls per
output tile (one per 3x3 tap, contracting all 256 input channels at once):
PSUM[o,pos] += sum_g sign(w)[g,i,o].T @ sign(x)[g,i, pos+tap_offset], on a
fully padded (58x58) image layout so every tap is a flat offset into the
same buffer. sign values are exact in fp8e4m3 and PSUM accumulates fp32,
so the conv is exact; the per-channel scale (mean |w|, computed on-device
in fp32) is applied on VectorE during PSUM eviction.

The sign-path inputs (x, and a transposed copy of w) are staged host-side
as bf16 — bf16 rounding never changes the sign of a float, so the device
sign() results are identical while DMA bytes halve. The scaling path
(w_nat) stays fp32 for an exact mean |w|.

Engine/queue split: x loads + sign on ScalarE (HW DMA queue), weight load
+ y stores on SyncE, scaling-path loads on GpSimd, scale-evict on VectorE.
"""

import numpy as np
import ml_dtypes

import concourse.bass as bass
import concourse.mybir as mybir
import concourse.tile as tile
from concourse import bacc
from concourse.bass import ds
from concourse.bass_utils import run_bass_kernel_spmd

N_CORES = 8
NIMG = 4            # images per core (32 / 8)
CIN = 256
COUT = 256
IG = 2              # input-channel groups of 128
OG = 2              # output-channel groups of 128
H = W = 56
HP = WP = 58        # padded spatial
PPI = HP * WP       # 3364 padded positions per image
NPOS = H * W        # 3136
T = 9               # 3x3 taps
RROWS = 8           # padded rows per output tile
RT = 7              # row tiles per image (rows 1..56)
NTILE = RROWS * WP  # 464 matmul free dim
NBLK = 1 + NIMG     # per-group: dummy margin block + 4 images
GLEN = NBLK * PPI + 76   # per-group flat length incl tail margin (16-mult)
XCH = 2             # x DMA/sign chunks per (image, group)
CROWS = H // XCH    # 28 rows per chunk

F32 = mybir.dt.float32
BF16 = mybir.dt.bfloat16
FP8 = mybir.dt.float8e4

_CACHE = {}


def build_nc():
    nc = bacc.Bacc("TRN2", target_bir_lowering=False, debug=False,
                   num_devices=N_CORES)

    x_d = nc.declare_dram_parameter("x", [NIMG, CIN, NPOS], BF16,
                                    isOutput=False)
    wn_d = nc.declare_dram_parameter("w_nat", [OG, 128, CIN * T], F32,
                                     isOutput=False)
    wt_d = nc.declare_dram_parameter("w_t", [T, IG, 128, COUT], BF16,
                                     isOutput=False)
    y_d = nc.declare_dram_parameter("y", [NIMG, COUT, NPOS], F32, isOutput=True)

    with tile.TileContext(nc) as tc:
        with (
            tc.tile_pool(name="persist", bufs=1) as persist,
            tc.tile_pool(name="xs", bufs=6) as xsp,
            tc.tile_pool(name="yb", bufs=4) as ybp,
            tc.tile_pool(name="ps", bufs=6, space=bass.MemorySpace.PSUM) as psp,
        ):
            ba = persist.tile([128, IG, GLEN], FP8)
            wsign = persist.tile([128, T, IG, COUT], FP8)
            wstage = persist.tile([128, T, IG, COUT], BF16)
            wnstage = persist.tile([128, OG, CIN * T], F32)
            scal = persist.tile([128, OG], F32)

            # binarized weights: DoubleRow lhsT tiles [i_in, (g, o)] per tap
            nc.sync.dma_start(
                out=wstage[:, :, :, :],
                in_=wt_d[:, :, :, :].rearrange("t g p o -> p t g o"))
            nc.scalar.sign(wsign[:, :, :, :], wstage[:, :, :, :])

            # zero padding borders (+ margins feeding discarded columns)
            ba4 = ba[:, :, 0:NBLK * PPI].rearrange(
                "p g (b h w) -> p g b h w", b=NBLK, h=HP, w=WP)
            nc.vector.memset(ba4[:, :, 0, HP - 1, :], 0.0)   # dummy tail row
            nc.vector.memset(ba[:, :, NBLK * PPI:], 0.0)     # tail margin
            nc.vector.memset(ba4[:, :, 1:NBLK, 0, :], 0.0)
            nc.vector.memset(ba4[:, :, 1:NBLK, HP - 1, :], 0.0)
            nc.vector.memset(ba4[:, :, 1:NBLK, :, 0], 0.0)
            nc.vector.memset(ba4[:, :, 1:NBLK, :, WP - 1], 0.0)

            # binarized activations, row-chunked so image 0 is ready fast
            for n in range(NIMG):
                for c in range(XCH):
                    for g in range(IG):
                        xs = xsp.tile([128, CROWS * W], BF16)
                        nc.scalar.dma_start(
                            out=xs[:, :],
                            in_=x_d[n, g * 128:(g + 1) * 128,
                                    ds(c * CROWS * W, CROWS * W)])
                        r0 = 1 + c * CROWS
                        nc.scalar.sign(
                            ba4[:, g, n + 1, r0:r0 + CROWS, 1:W + 1],
                            xs[:, :].rearrange("p (h w) -> p h w",
                                               h=CROWS, w=W))
                if n == 0:
                    # scaling path: fp32 |w| mean, needed ~20us in
                    for q in range(OG):
                        nc.gpsimd.dma_start(out=wnstage[:, q, :],
                                            in_=wn_d[q])
                        nc.vector.tensor_reduce(
                            out=scal[:, q:q + 1], in_=wnstage[:, q, :],
                            axis=mybir.AxisListType.X, op=mybir.AluOpType.add,
                            apply_absolute_value=True)
                    nc.vector.tensor_scalar_mul(scal[:, :], scal[:, :],
                                                1.0 / (CIN * T))

            # main conv loop
            for n in range(NIMG):
                for r in range(RT):
                    h0 = 1 + r * RROWS
                    for q in range(OG):
                        ps = psp.tile([128, NTILE], F32)
                        for t in range(T):
                            kh, kw = t // 3, t % 3
                            off = ((n + 1) * PPI
                                   + (h0 + kh - 1) * WP + (kw - 1))
                            nc.tensor.matmul(
                                ps[:, :],
                                wsign[:, t, :, q * 128:(q + 1) * 128],
                                ba[:, :, ds(off, NTILE)],
                                start=(t == 0), stop=(t == T - 1),
                                perf_mode=mybir.MatmulPerfMode.DoubleRow)
                        yb = ybp.tile([128, RROWS, W], F32)
                        ps3 = ps[:, :].rearrange("p (h w) -> p h w",
                                                 h=RROWS, w=WP)
                        nc.vector.tensor_scalar_mul(
                            yb[:, :, :], ps3[:, :, 1:W + 1], scal[:, q:q + 1])
                        nc.sync.dma_start(
                            out=y_d[n, q * 128:(q + 1) * 128,
                                    ds((h0 - 1) * W, RROWS * W)],
                            in_=yb[:, :, :])

    nc.compile()
    return nc


def _get_nc():
    if "nc" not in _CACHE:
        _CACHE["nc"] = build_nc()
    return _CACHE["nc"]


def _prep_inputs(x, weight):
    x = np.ascontiguousarray(x, dtype=np.float32)
    weight = np.ascontiguousarray(weight, dtype=np.float32)
    # natural layout (fp32) for the exact |w| reduction: (o_grp, o_in, i*9)
    w_nat = weight.reshape(OG, 128, CIN * T)
    # transposed sign-path copy in bf16: (tap, i_grp, i_in, o)
    w_t = np.ascontiguousarray(
        weight.transpose(2, 3, 1, 0).reshape(T, IG, 128, COUT)
    ).astype(ml_dtypes.bfloat16)
    xb = x.astype(ml_dtypes.bfloat16)
    in_maps = []
    for c in range(N_CORES):
        xs = xb[c * NIMG:(c + 1) * NIMG].reshape(NIMG, CIN, NPOS)
        in_maps.append({"x": np.ascontiguousarray(xs),
                        "w_nat": w_nat, "w_t": w_t})
    return in_maps


def _assemble(results):
    parts = [results[c]["y"].reshape(NIMG, COUT, H, W) for c in range(N_CORES)]
    return np.concatenate(parts, axis=0)


def run(x, weight, **run_kwargs):
    nc = _get_nc()
    in_maps = _prep_inputs(x, weight)
    res = run_bass_kernel_spmd(nc, in_maps, list(range(N_CORES)), **run_kwargs)
    return _assemble(res.results), res


def kernel(x, weight):
    y, _ = run(x, weight)
    return y


# revision 6
# speedup vs baseline: 1.8543x; 1.1893x over previous
"""HardBinaryConv forward on 8 trn2 NeuronCores.

y = conv2d(sign(x), scaling[o] * sign(w)), 3x3, pad 1, stride 1
  x: (32, 256, 56, 56) f32, w: (256, 256, 3, 3) f32
  scaling[o] = mean(|w[o]|)

Strategy: data-parallel over batch (4 images / core), weights replicated.
Per core the conv is computed as 9 accumulating fp8 DoubleRow matmuls per
output tile (one per 3x3 tap, contracting all 256 input channels at once):
PSUM[o,pos] += sum_g sign(w)[g,i,o].T @ sign(x)[g,i, pos+tap_offset], on a
fully padded (58x58) image layout so every tap is a flat offset into the
same buffer. sign values are exact in fp8e4m3 and PSUM accumulates fp32,
so the conv is exact; the per-channel scale (mean |w|, computed on-device
in fp32) is applied on VectorE during PSUM eviction.

Layout detail: the two 128-channel groups of one image sit in adjacent
GLEN_I blocks so a DoubleRow rhs AP [128, 2, 464] only spans that image
(keeps Tile's bounding-box deps tight); each block has 1 head + 11 tail
margin elements absorbing the +-1 tap offsets of border columns (those
products land only in discarded padded-column outputs).

The sign-path inputs (x, and a transposed copy of w) are staged host-side
as bf16 — bf16 rounding never changes the sign of a float, so the device
sign() results are identical while DMA bytes halve. The scaling path
(w_nat) stays fp32 for an exact mean |w|.

Engine/queue split: x loads + sign on ScalarE (HW DMA queue), weight load
+ y stores on SyncE, scaling-path loads on GpSimd, scale-evict on VectorE.
"""

import numpy as np
import ml_dtypes

import concourse.bass as bass
import concourse.mybir as mybir
import concourse.tile as tile
from concourse import bacc
from concourse.bass import ds
from concourse.bass_utils import run_bass_kernel_spmd

N_CORES = 8
NIMG = 4            # images per core (32 / 8)
CIN = 256
COUT = 256
IG = 2              # input-channel groups of 128
OG = 2              # output-channel groups of 128
H = W = 56
HP = WP = 58        # padded spatial
PPI = HP * WP       # 3364 padded positions per image
NPOS = H * W        # 3136
T = 9               # 3x3 taps
RROWS = 8           # padded rows per output tile
RT = 7              # row tiles per image (rows 1..56)
NTILE = RROWS * WP  # 464 matmul free dim
GLEN_I = 3376       # per-(image,group) block: 1 head + 3364 + 11 tail
IMGOFF = 1          # image data offset inside its block
XCH = 2             # x DMA/sign chunks per (image, group)
CROWS = H // XCH    # 28 rows per chunk

F32 = mybir.dt.float32
BF16 = mybir.dt.bfloat16
FP8 = mybir.dt.float8e4

_CACHE = {}


def build_nc():
    nc = bacc.Bacc("TRN2", target_bir_lowering=False, debug=False,
                   num_devices=N_CORES)

    x_d = nc.declare_dram_parameter("x", [NIMG, CIN, NPOS], BF16,
                                    isOutput=False)
    wn_d = nc.declare_dram_parameter("w_nat", [OG, 128, CIN * T], F32,
                                     isOutput=False)
    wt_d = nc.declare_dram_parameter("w_t", [128, T, IG, COUT], BF16,
                                     isOutput=False)
    y_d = nc.declare_dram_parameter("y", [NIMG, COUT, NPOS], F32, isOutput=True)

    with tile.TileContext(nc) as tc:
        with (
            tc.tile_pool(name="persist", bufs=1) as persist,
            tc.tile_pool(name="xs", bufs=8) as xsp,
            tc.tile_pool(name="yb", bufs=4) as ybp,
            tc.tile_pool(name="ps", bufs=6, space=bass.MemorySpace.PSUM) as psp,
        ):
            ba = persist.tile([128, NIMG, IG, GLEN_I], FP8)
            wsign = persist.tile([128, T, IG, COUT], FP8)
            wstage = persist.tile([128, T, IG, COUT], BF16)
            wnstage = persist.tile([128, OG, CIN * T], F32)
            scal = persist.tile([128, OG], F32)

            def seg(n, g):
                """58x58 padded-image view of block (n, g)."""
                return ba[:, n, g, IMGOFF:IMGOFF + PPI].rearrange(
                    "p (h w) -> p h w", h=HP, w=WP)

            # binarized weights: DoubleRow lhsT tiles [i_in, (g, o)] per
            # tap; host sends them partition-major so this is one clean
            # contiguous DMA, and the sign is split for earlier readiness
            nc.sync.dma_start(out=wstage[:, :, :, :], in_=wt_d[:, :, :, :])
            nc.scalar.sign(wsign[:, 0:4, :, :], wstage[:, 0:4, :, :])
            nc.scalar.sign(wsign[:, 4:T, :, :], wstage[:, 4:T, :, :])

            # zero padding borders + margins feeding discarded columns
            nc.vector.memset(ba[:, :, :, 0:IMGOFF], 0.0)
            nc.vector.memset(ba[:, :, :, IMGOFF + PPI:], 0.0)
            for n in range(NIMG):
                for g in range(IG):
                    s = seg(n, g)
                    nc.vector.memset(s[:, 0, :], 0.0)
                    nc.vector.memset(s[:, HP - 1, :], 0.0)
                    nc.vector.memset(s[:, :, 0], 0.0)
                    nc.vector.memset(s[:, :, WP - 1], 0.0)

            # binarized activations, row-chunked; triggers issued ahead of
            # signs so the DMA pipeline stays deep
            for n in range(NIMG):
                chunks = []
                for c in range(XCH):
                    for g in range(IG):
                        xs = xsp.tile([128, CROWS * W], BF16)
                        nc.scalar.dma_start(
                            out=xs[:, :],
                            in_=x_d[n, g * 128:(g + 1) * 128,
                                    ds(c * CROWS * W, CROWS * W)])
                        chunks.append((c, g, xs))
                for c, g, xs in chunks:
                    r0 = 1 + c * CROWS
                    nc.scalar.sign(
                        seg(n, g)[:, r0:r0 + CROWS, 1:W + 1],
                        xs[:, :].rearrange("p (h w) -> p h w", h=CROWS, w=W))
                if n == 0:
                    # scaling path: fp32 |w| mean, needed ~20us in
                    for q in range(OG):
                        nc.gpsimd.dma_start(out=wnstage[:, q, :],
                                            in_=wn_d[q])
                        nc.vector.tensor_reduce(
                            out=scal[:, q:q + 1], in_=wnstage[:, q, :],
                            axis=mybir.AxisListType.X, op=mybir.AluOpType.add,
                            apply_absolute_value=True)
                    nc.vector.tensor_scalar_mul(scal[:, :], scal[:, :],
                                                1.0 / (CIN * T))

            # main conv loop
            for n in range(NIMG):
                for r in range(RT):
                    h0 = 1 + r * RROWS
                    for q in range(OG):
                        ps = psp.tile([128, NTILE], F32)
                        for t in range(T):
                            kh, kw = t // 3, t % 3
                            off = IMGOFF + (h0 + kh - 1) * WP + (kw - 1)
                            nc.tensor.matmul(
                                ps[:, :],
                                wsign[:, t, :, q * 128:(q + 1) * 128],
                                ba[:, n, :, ds(off, NTILE)],
                                start=(t == 0), stop=(t == T - 1),
                                perf_mode=mybir.MatmulPerfMode.DoubleRow)
                        yb = ybp.tile([128, RROWS, W], F32)
                        ps3 = ps[:, :].rearrange("p (h w) -> p h w",
                                                 h=RROWS, w=WP)
                        nc.vector.tensor_scalar_mul(
                            yb[:, :, :], ps3[:, :, 1:W + 1], scal[:, q:q + 1])
                        nc.sync.dma_start(
                            out=y_d[n, q * 128:(q + 1) * 128,
                                    ds((h0 - 1) * W, RROWS * W)],
                            in_=yb[:, :, :])

    nc.compile()
    return nc


def _get_nc():
    if "nc" not in _CACHE:
        _CACHE["nc"] = build_nc()
    return _CACHE["nc"]


def _prep_inputs(x, weight):
    x = np.ascontiguousarray(x, dtype=np.float32)
    weight = np.ascontiguousarray(weight, dtype=np.float32)
    # natural layout (fp32) for the exact |w| reduction: (o_grp, o_in, i*9)
    w_nat = weight.reshape(OG, 128, CIN * T)
    # partition-major sign-path copy in bf16: (i_in, tap, i_grp, o)
    w_t = np.ascontiguousarray(
        weight.reshape(COUT, IG, 128, T).transpose(2, 3, 1, 0)
    ).astype(ml_dtypes.bfloat16)
    xb = x.astype(ml_dtypes.bfloat16)
    in_maps = []
    for c in range(N_CORES):
        xs = xb[c * NIMG:(c + 1) * NIMG].reshape(NIMG, CIN, NPOS)
        in_maps.append({"x": np.ascontiguousarray(xs),
                        "w_nat": w_nat, "w_t": w_t})
    return in_maps


def _assemble(results):
    parts = [results[c]["y"].reshape(NIMG, COUT, H, W) for c in range(N_CORES)]
    return np.concatenate(parts, axis=0)


def run(x, weight, **run_kwargs):
    nc = _get_nc()
    in_maps = _prep_inputs(x, weight)
    res = run_bass_kernel_spmd(nc, in_maps, list(range(N_CORES)), **run_kwargs)
    return _assemble(res.results), res


def kernel(x, weight):
    y, _ = run(x, weight)
    return y
